# revision 15
# baseline (speedup 1.0000x reference)
"""Cross-attention block on 8 Trainium2 NeuronCores.

Computes, per batch b:
    xn = LN(x); cn = LN(cond)
    q = xn @ Wq; k = cn @ Wk; v = cn @ Wv   (8 heads x 64)
    out = softmax(q k^T / sqrt(64)) v
    y  = LN(out @ Wo + bo + x)

Sharding: 8 cores = 4 batches x 2 query-row halves (data parallel over
(batch, query-block)).  Each core recomputes LN(cond)/K/V for its batch
(duplicated across the 2 cores of a batch) and produces a disjoint
[1024, 512] slice of the output, so no collectives are needed.

v2 structure (vs the v1 baseline at 377us):
 - Phase A fuses LN -> PE-transpose -> Q/K/V projections per 512-token
   group, so the tensor engine has no idle gap longer than the HAM
   re-throttle window (3.4us) and runs at 2.4 GHz instead of 1.2.
 - Weights are cast to bf16 on the host, halving weight DMA and
   removing the on-device cast.
 - Score matmuls write bf16 directly to PSUM, so one exp ACTIVATE
   covers 2048 elements/partition (64 exps instead of 128).
 - PSUM ring: score tiles 2x2 banks + a shared 4x1-bank f32
   accumulator tag (projections, attention-out, Wo), sized to exactly
   8 banks; attention-out double buffering across head-pair blocks
   removes the per-block PE stall that re-throttled the clock.
 - Softmax denominators use reciprocal_approx_fast (~5x faster).
 - Wo + residual + LN stats for the first query half run inside the
   attention phase; only the final sqrt/scale runs as a tail.
"""

import functools

import numpy as np

B, N, M = 4, 2048, 2048
DQ, DC = 512, 768
H, DH = 8, 64
INNER = H * DH  # 512
P = 128
NQ = N // 2  # query rows per core
EPS = 1e-5
N_CORES = 8

FC_X = DQ // P  # 4 feature chunks of x
FC_C = DC // P  # 6 feature chunks of cond
IC = INNER // P  # 4 inner chunks
TQ = NQ // P  # 8 query-token chunks per core
TK = M // P  # 16 key-token chunks
NT = NQ // 512  # 2 query column tiles (transposed layout)
CG = M // 512  # 4 cond token groups


def _emit(tc, io):
    import contextlib

    import concourse.bass as bass
    import concourse.mybir as mybir

    nc = tc.nc
    f32 = mybir.dt.float32
    bf16 = mybir.dt.bfloat16
    AF = mybir.ActivationFunctionType
    OP = mybir.AluOpType

    ctx = contextlib.ExitStack()
    with ctx:
        singles = ctx.enter_context(tc.tile_pool(name="singles", bufs=1))
        work = ctx.enter_context(tc.tile_pool(name="work", bufs=3))
        stat = ctx.enter_context(tc.tile_pool(name="stat", bufs=4))
        cenp = ctx.enter_context(tc.tile_pool(name="cenp", bufs=5))
        ppool = ctx.enter_context(tc.tile_pool(name="ppool", bufs=3))
        ps = ctx.enter_context(tc.tile_pool(name="ps", bufs=2, space="PSUM"))

        # ---- constants -------------------------------------------------
        from concourse.masks import make_identity

        ident = singles.tile([P, P], bf16, name="ident")
        make_identity(nc, ident)
        eps_t = singles.tile([P, 1], f32, name="eps_t")
        nc.vector.memset(eps_t, EPS)

        def bcast_load(vec_ap, width, name):
            """[width] dram vector -> [128, width] sbuf tile (same row on
            every partition)."""
            t = singles.tile([P, width], f32, name=name)
            bc = bass.AP(
                tensor=vec_ap.tensor,
                offset=vec_ap.offset,
                ap=[[0, P]] + [list(a) for a in vec_ap.ap],
            )
            nc.gpsimd.dma_start(out=t, in_=bc)
            return t

        def strip_load(vec_ap, chunks, name):
            """[chunks*128] dram vector -> [128, chunks] sbuf (feature-on-
            partition layout)."""
            t = singles.tile([P, chunks], f32, name=name)
            nc.sync.dma_start(out=t, in_=vec_ap.rearrange("(c p) -> p c", p=P))
            return t

        gx = strip_load(io["lnx_g"], FC_X, "gx")
        bx = strip_load(io["lnx_b"], FC_X, "bx")
        gc = strip_load(io["lnc_g"], FC_C, "gc")
        bc_ = strip_load(io["lnc_b"], FC_C, "bc")

        # ---- weights: bf16 in HBM (host-cast), contraction on partitions.
        # DMAs for these are emitted inside the phase-A loop, ordered so the
        # first cond token group lands before the weights hog the queue.
        wq_b = singles.tile([P, FC_X, INNER], bf16, name="wq_b")
        wk_b = singles.tile([P, FC_C, INNER], bf16, name="wk_b")
        wv_b = singles.tile([P, FC_C, INNER], bf16, name="wv_b")
        # Wo in head-major rows to match the 64-partition O^T layout.
        wo_b = singles.tile([DH, H, DQ], bf16, name="wo_b")

        # ---- persistent activations ------------------------------------
        xnT = singles.tile([P, FC_X, NQ], bf16, name="xnT")  # LN(x)^T
        cnT = singles.tile([P, FC_C, M], bf16, name="cnT")  # LN(cond)^T
        QT = singles.tile([P, IC, NQ], bf16, name="QT")  # (q*scale)^T
        KT = singles.tile([P, IC, M], bf16, name="KT")  # k^T
        # v in token layout, one ones-column per head for the fused
        # softmax denominator: V_sb[:, mc, h, 0:64] = v, [..., 64] = 1.
        V_sb = singles.tile([P, TK, H, DH + 1], bf16, name="V_sb")
        nc.vector.memset(V_sb[:, :, :, DH : DH + 1], 1.0)
        # attn out^T, head-major on 64 partitions
        OT = singles.tile([DH, H, NQ], bf16, name="OT")
        # x residual tiles (bo gets folded in during phase B) and the
        # pre-normalize final output accumulator
        xres = singles.tile([P, TQ, DQ], f32, name="xres")
        xb = singles.tile([P, TQ, DQ], f32, name="xb")  # x + bo
        y1s = singles.tile([P, TQ, DQ], f32, name="y1s")
        mvs = singles.tile([P, TQ, 2], f32, name="mvs")

        # ---- phase A: LN + PE transpose + projections, per token group --
        # PSUM tags: "st" ([P,4,512] bf16 = 2 banks, 2 bufs) shared by the
        # phase-A transposes and the phase-B score tiles; "acc"
        # ([P,512] f32 = 1 bank, 4 bufs) shared by projection / attention-
        # out / Wo accumulators.  4 + 4 = 8 banks exactly.
        def ln_group(src, width, tg, g_strip, b_strip, dst):
            import math

            fmax = math.gcd(512, width)
            nsub = width // fmax
            fc_n = width // P
            cents, mvl = [], []
            std4 = stat.tile([P, 4], f32, tag="std", name="std4")
            for tl in range(4):
                t = tg * 4 + tl
                if src is None:  # x: already resident in xres
                    x_t = xres[:, t]
                else:
                    x_t = work.tile([P, width], f32, tag="xin", bufs=4, name="x_t")
                    nc.sync.dma_start(out=x_t, in_=src[:, t])
                if nsub == 1:
                    stats = stat.tile([P, 6], f32, tag="bnstats", bufs=6, name="st6")
                    nc.vector.bn_stats(out=stats, in_=x_t)
                else:
                    xr = x_t.rearrange("p (s f) -> p s f", f=fmax)
                    stats = stat.tile(
                        [P, nsub, 6], f32, tag="bnstats", bufs=6, name="st6"
                    )
                    for s in range(nsub):
                        nc.vector.bn_stats(out=stats[:, s], in_=xr[:, s])
                mv = stat.tile([P, 2], f32, tag="bnaggr", bufs=6, name="mv")
                nc.vector.bn_aggr(out=mv, in_=stats)
                nc.scalar.activation(
                    out=std4[:, tl : tl + 1],
                    in_=mv[:, 1:2],
                    func=AF.Sqrt,
                    bias=eps_t,
                    scale=1.0,
                )
                cents.append(x_t)
                mvl.append(mv)
            rstd = stat.tile([P, 4], f32, tag="rstd", name="rstd")
            nc.vector.reciprocal(out=rstd, in_=std4)
            nmr = stat.tile([P, 4], f32, tag="nmr", name="nmr")
            for tl in range(4):
                nc.vector.scalar_tensor_tensor(
                    out=nmr[:, tl : tl + 1],
                    in0=mvl[tl][:, 0:1],
                    scalar=-1.0,
                    in1=rstd[:, tl : tl + 1],
                    op0=OP.mult,
                    op1=OP.mult,
                )
            cen_ts = []
            for tl in range(4):
                cen = cenp.tile([P, width], bf16, tag="cen", name="cen")
                # (x - mean) * rstd on the scalar engine (idle in phase A)
                nc.scalar.activation(
                    out=cen,
                    in_=cents[tl],
                    func=AF.Identity,
                    bias=nmr[:, tl : tl + 1],
                    scale=rstd[:, tl : tl + 1],
                )
                cen_ts.append(cen)
            for fc in range(fc_n):
                tp = ps.tile([P, 4, P], bf16, tag="st", bufs=2, name="tp")
                for tl in range(4):
                    nc.tensor.transpose(
                        tp[:, tl], cen_ts[tl][:, fc * P : (fc + 1) * P], ident
                    )
                # dst = tp * g[fc] + b[fc]   (per-partition scalars)
                nc.vector.tensor_scalar(
                    out=dst[:, fc, tg * 512 : (tg + 1) * 512],
                    in0=tp,
                    scalar1=g_strip[:, fc : fc + 1],
                    scalar2=b_strip[:, fc : fc + 1],
                    op0=OP.mult,
                    op1=OP.add,
                )

        condr = io["cond"].rearrange("(t p) d -> p t d", p=P)
        xr = io["x"].rearrange("(t p) d -> p t d", p=P)

        scale = float(DH) ** -0.5
        for tg in range(CG):
            ln_group(condr, DC, tg, gc, bc_, cnT)
            if tg == 0:
                # Queue the weight loads behind the first cond group.
                nc.sync.dma_start(
                    out=wk_b, in_=io["Wk"].rearrange("(ko p) i -> p ko i", p=P)
                )
                nc.sync.dma_start(
                    out=wv_b, in_=io["Wv"].rearrange("(ko p) i -> p ko i", p=P)
                )
            if tg == 1:
                nc.sync.dma_start(
                    out=wq_b, in_=io["Wq"].rearrange("(ko p) i -> p ko i", p=P)
                )
                nc.sync.dma_start(
                    out=wo_b, in_=io["Wo"].rearrange("(h p) d -> p h d", p=DH)
                )
                # x tiles land in their long-lived residual slots.
                for t in range(TQ):
                    nc.sync.dma_start(out=xres[:, t], in_=xr[:, t])
            sl = slice(tg * 512, (tg + 1) * 512)
            # KT columns for this token group
            for m in range(IC):
                kps = ps.tile([P, 512], f32, tag="acc", bufs=4, name="kps")
                for k in range(FC_C):
                    nc.tensor.matmul(
                        kps,
                        lhsT=wk_b[:, k, m * P : (m + 1) * P],
                        rhs=cnT[:, k, sl],
                        start=(k == 0),
                        stop=(k == FC_C - 1),
                    )
                nc.scalar.copy(out=KT[:, m, sl], in_=kps)
            # V rows for this token group
            for mc in range(tg * 4, tg * 4 + 4):
                vps = ps.tile([P, 512], f32, tag="acc", bufs=4, name="vps")
                for k in range(FC_C):
                    nc.tensor.matmul(
                        vps,
                        lhsT=cnT[:, k, mc * P : (mc + 1) * P],
                        rhs=wv_b[:, k, :],
                        start=(k == 0),
                        stop=(k == FC_C - 1),
                    )
                nc.scalar.copy(
                    out=V_sb[:, mc, :, 0:DH],
                    in_=vps.rearrange("p (h d) -> p h d", h=H),
                )

        gf_bc = bcast_load(io["lnf_g"], DQ, "gf_bc")
        bf_bc = bcast_load(io["lnf_b"], DQ, "bf_bc")
        bo_bc = bcast_load(io["bo"], DQ, "bo_bc")

        for xg in range(NT):
            ln_group(None, DQ, xg, gx, bx, xnT)
            sl = slice(xg * 512, (xg + 1) * 512)
            for m in range(IC):
                qps = ps.tile([P, 512], f32, tag="acc", bufs=4, name="qps")
                for k in range(FC_X):
                    nc.tensor.matmul(
                        qps,
                        lhsT=wq_b[:, k, m * P : (m + 1) * P],
                        rhs=xnT[:, k, sl],
                        start=(k == 0),
                        stop=(k == FC_X - 1),
                    )
                nc.scalar.activation(
                    out=QT[:, m, sl], in_=qps, func=AF.Copy, scale=scale
                )

        # Fold bo into the residual tiles (gpsimd; runs during phase B).
        for t in range(TQ):
            nc.gpsimd.tensor_add(out=xb[:, t], in0=xres[:, t], in1=bo_bc)

        # Dummy exp: pulls the exp table-set load off phase B's critical path.
        dummy = stat.tile([1, 1], f32, tag="dummy", bufs=1, name="dummy")
        nc.scalar.activation(out=dummy, in_=eps_t[0:1, 0:1], func=AF.Exp)

        # ---- phase B/C: attention + Wo/residual per query tile ----------
        def emit_block(nt, c):
            hA, hB = 2 * c, 2 * c + 1
            ot = {
                h: ps.tile([P, 512], f32, tag="acc", bufs=4, name=f"ot{h % 2}")
                for h in (hA, hB)
            }
            sl = slice(nt * 512, (nt + 1) * 512)
            q_a = QT[0:DH, c, sl]
            q_b = QT[DH:P, c, sl]

            def emit_pv(mc, p):
                for j, h in enumerate((hA, hB)):
                    nc.tensor.matmul(
                        ot[h][0 : DH + 1, :],
                        lhsT=V_sb[:, mc, h, :],
                        rhs=p[:, j],
                        start=(mc == 0),
                        stop=(mc == TK - 1),
                    )

            # st slot j = head j, so consecutive score matmuls alternate
            # PE row groups (h0 / h64) and LDWEIGHTS pulls ahead.
            pend = None
            for mc in range(TK):
                st = ps.tile([P, 2, 512], f32, tag="st", bufs=2, name="stb")
                nc.tensor.matmul(
                    st[:, 0],
                    lhsT=KT[0:DH, c, mc * P : (mc + 1) * P],
                    rhs=q_a,
                    start=True,
                    stop=True,
                )
                nc.tensor.matmul(
                    st[:, 1],
                    lhsT=KT[DH:P, c, mc * P : (mc + 1) * P],
                    rhs=q_b,
                    start=True,
                    stop=True,
                )
                p = ppool.tile([P, 2, 512], bf16, tag="p", name="p")
                nc.scalar.activation(out=p, in_=st, func=AF.Exp)
                if pend is not None:
                    emit_pv(*pend)
                pend = (mc, p)
            emit_pv(*pend)

            # normalize: row DH of ot[h] holds the softmax denominator.
            for h in (hA, hB):
                rb = work.tile([P, 512], f32, tag="rb", bufs=2, name="rb")
                nc.vector.reciprocal(
                    out=rb[DH : DH + 1, :], in_=ot[h][DH : DH + 1, :]
                )
                r0 = work.tile([1, 512], f32, tag="r0", bufs=2, name="r0")
                nc.sync.dma_start(out=r0, in_=rb[DH : DH + 1, :])
                nc.gpsimd.partition_broadcast(rb[0:DH, :], r0[0:1, :])
                nc.vector.tensor_mul(
                    out=OT[:, h, nt * 512 : (nt + 1) * 512],
                    in0=ot[h][0:DH, :],
                    in1=rb[0:DH, :],
                )

        def emit_wo(ts):
            # Wo + residual + LN stats for these query tiles; the sqrt and
            # final scale run later so the exp table set stays loaded.
            for t in ts:
                y_ps = ps.tile([P, 512], f32, tag="acc", bufs=4, name="y_ps")
                for h in range(H):
                    nc.tensor.matmul(
                        y_ps,
                        lhsT=OT[:, h, t * P : (t + 1) * P],
                        rhs=wo_b[:, h, :],
                        start=(h == 0),
                        stop=(h == H - 1),
                    )
                nc.vector.tensor_add(out=y1s[:, t], in0=y_ps, in1=xb[:, t])
                stats = stat.tile([P, 6], f32, tag="bnstats", bufs=6, name="stf")
                nc.vector.bn_stats(out=stats, in_=y1s[:, t])
                nc.vector.bn_aggr(out=mvs[:, t], in_=stats)

        outr = io["out"].rearrange("(t p) d -> p t d", p=P)

        def final_apply(ts):
            # sqrt + scale + store for a set of query tiles (their Wo /
            # residual / stats must already be emitted).
            n = len(ts)
            stdn = stat.tile([P, n], f32, tag="stdn", bufs=2, name="stdn")
            for i, t in enumerate(ts):
                nc.scalar.activation(
                    out=stdn[:, i : i + 1],
                    in_=mvs[:, t, 1:2],
                    func=AF.Sqrt,
                    bias=eps_t,
                    scale=1.0,
                )
            rstdn = stat.tile([P, n], f32, tag="rstdn", bufs=2, name="rstdn")
            nc.vector.reciprocal(out=rstdn, in_=stdn)
            for i, t in enumerate(ts):
                y1 = y1s[:, t]
                nc.vector.tensor_scalar(
                    out=y1,
                    in0=y1,
                    scalar1=mvs[:, t, 0:1],
                    scalar2=rstdn[:, i : i + 1],
                    op0=OP.subtract,
                    op1=OP.mult,
                )
                nc.vector.tensor_mul(out=y1, in0=y1, in1=gf_bc)
                nc.gpsimd.tensor_add(out=y1, in0=y1, in1=bf_bc)
                nc.sync.dma_start(out=outr[:, t], in_=y1)

        # nt=0's Wo/residual work is spread across nt=1's blocks (one or
        # two query tiles per boundary, so the shared PSUM accumulator ring
        # never makes a Wo matmul wait on a fresh block's normalize); its
        # final LN apply (one sqrt table switch) hides under later blocks'
        # exps.
        blocks = [(nt, c) for nt in range(NT) for c in range(H // 2)]
        for i, (nt, c) in enumerate(blocks):
            emit_block(nt, c)
            if (nt, c) == (1, 0):
                emit_wo([0])
            elif (nt, c) == (1, 1):
                emit_wo([1, 2])
            elif (nt, c) == (1, 2):
                emit_wo([3])
                final_apply([0, 1, 2, 3])
        emit_wo([4, 5, 6, 7])
        final_apply([4, 5, 6, 7])

        if "p_cnT" in io:
            nc.sync.dma_start(out=io["p_cnT"], in_=cnT)
            nc.sync.dma_start(out=io["p_xnT"], in_=xnT)
            nc.sync.dma_start(out=io["p_QT"], in_=QT)
            nc.sync.dma_start(out=io["p_KT"], in_=KT)
            nc.sync.dma_start(out=io["p_V"], in_=V_sb)
            nc.sync.dma_start(out=io["p_OT"], in_=OT)



@functools.cache
def _build_program():
    global PROBE
    import concourse.bacc as bacc
    import concourse.mybir as mybir
    import concourse.tile as tile

    f32 = mybir.dt.float32
    bf16 = mybir.dt.bfloat16
    nc = bacc.Bacc()
    io = {}
    io["x"] = nc.declare_dram_parameter("x", [NQ, DQ], f32, False)[:, :]
    io["cond"] = nc.declare_dram_parameter("cond", [M, DC], f32, False)[:, :]
    for name in ("lnx_g", "lnx_b"):
        io[name] = nc.declare_dram_parameter(name, [DQ], f32, False)[:]
    for name in ("lnc_g", "lnc_b"):
        io[name] = nc.declare_dram_parameter(name, [DC], f32, False)[:]
    io["Wq"] = nc.declare_dram_parameter("Wq", [DQ, INNER], bf16, False)[:, :]
    io["Wk"] = nc.declare_dram_parameter("Wk", [DC, INNER], bf16, False)[:, :]
    io["Wv"] = nc.declare_dram_parameter("Wv", [DC, INNER], bf16, False)[:, :]
    io["Wo"] = nc.declare_dram_parameter("Wo", [INNER, DQ], bf16, False)[:, :]
    for name in ("bo", "lnf_g", "lnf_b"):
        io[name] = nc.declare_dram_parameter(name, [DQ], f32, False)[:]
    io["out"] = nc.declare_dram_parameter("out", [NQ, DQ], f32, True)[:, :]
    if PROBE:
        bf = bf16
        io["p_cnT"] = nc.declare_dram_parameter("p_cnT", [P, FC_C, M], bf, True)[:, :, :]
        io["p_xnT"] = nc.declare_dram_parameter("p_xnT", [P, FC_X, NQ], bf, True)[:, :, :]
        io["p_QT"] = nc.declare_dram_parameter("p_QT", [P, IC, NQ], bf, True)[:, :, :]
        io["p_KT"] = nc.declare_dram_parameter("p_KT", [P, IC, M], bf, True)[:, :, :]
        io["p_V"] = nc.declare_dram_parameter("p_V", [P, TK, H, DH + 1], bf, True)[:, :, :, :]
        io["p_OT"] = nc.declare_dram_parameter("p_OT", [DH, H, NQ], bf, True)[:, :, :]

    with tile.TileContext(nc) as tc:
        _emit(tc, io)
    nc.compile()
    return nc


def _core_input_map(inputs, core):
    import ml_dtypes

    b, half = core // 2, core % 2
    m = {
        "x": np.ascontiguousarray(inputs["x"][b, half * NQ : (half + 1) * NQ]),
        "cond": np.ascontiguousarray(inputs["cond"][b]),
    }
    for name in ("lnx_g", "lnx_b", "lnc_g", "lnc_b", "bo", "lnf_g", "lnf_b"):
        m[name] = np.asarray(inputs[name], dtype=np.float32)
    for name in ("Wq", "Wk", "Wv", "Wo"):
        m[name] = np.asarray(inputs[name]).astype(ml_dtypes.bfloat16)
    return m


TRACE = False
PROBE = False
LAST_RESULTS = None


def kernel(**inputs):
    from concourse.bass_utils import run_bass_kernel_spmd

    global LAST_RESULTS
    nc = _build_program()
    in_maps = [_core_input_map(inputs, core) for core in range(N_CORES)]
    res = run_bass_kernel_spmd(
        nc,
        in_maps,
        list(range(N_CORES)),
        trace=TRACE,
        trace_cores=[0] if TRACE else None,
    )
    LAST_RESULTS = res
    out = np.empty((B, N, DQ), np.float32)
    for core in range(N_CORES):
        b, half = core // 2, core % 2
        out[b, half * NQ : (half + 1) * NQ] = res.results[core]["out"]
    return out
